# revision 1
# baseline (speedup 1.0000x reference)
"""MedianPool2d 3x3 stride-1 reflect-pad kernel for 8 TRN2 NeuronCores.

Input:  x [16, 3, 512, 512] fp32 (full). Output: same shape, lower median
of each 3x3 window after reflect pad (exact order statistic -> bitwise
exact vs reference).

Strategy:
 - Pure data parallel: 48 images (B*C) -> 6 images per core.
 - Host-side staging: per core, each of 3 tiles holds 2 images split
   across 128 partitions; partition p carries 8 output rows plus its
   2 halo rows and reflect-padded columns, flattened to 10*514 fp32.
   Both vertical (+-514) and horizontal (+-1) window shifts then become
   free-dim offsets of one flat SBUF buffer, and each tile needs exactly
   one input DMA.
 - Median-of-9 via the exact identity
       med9 = med3( max3(col mins), med3(col meds), min3(col maxes) )
   computed with 18 full-tile min/max tensor_tensor ops, statically
   split across DVE (10) and GpSimd/Pool (8).
"""

import sys

for _p in ("/opt/trn_rl_repo", "/root/.axon_site/_ro/trn_rl_repo"):
    if _p not in sys.path:
        sys.path.append(_p)

import numpy as np

import concourse.bass as bass
import concourse.bacc as bacc
import concourse.mybir as mybir
from concourse.tile import TileContext

F32 = mybir.dt.float32
MIN = mybir.AluOpType.min
MAX = mybir.AluOpType.max

ROWS_PER_CORE = 3072  # 6 images x 512 rows
W = 512
WP = 514  # padded row width
RPP = 8  # image rows per partition
NSLOT = RPP + 2  # + top/bottom halo rows
FLAT = NSLOT * WP  # 5140 floats per partition in the staged input
CLEN = RPP * WP  # 4112: flat length of per-position stats / outputs
N_TILES = 3  # 2 images per tile
ROWS_PER_TILE = 1024

_NC_CACHE = None


def _build_bass(loop_k=1):
    nc = bacc.Bacc("TRN2", target_bir_lowering=False)
    x_d = nc.declare_dram_parameter("x", [N_TILES, 128, FLAT], F32, isOutput=False)
    o_d = nc.declare_dram_parameter("out", [ROWS_PER_CORE, W], F32, isOutput=True)

    import contextlib
    with TileContext(nc) as tc:
        loop_cm = tc.For_i(0, loop_k, 1) if loop_k > 1 else contextlib.nullcontext()
        with loop_cm, tc.tile_pool(name="pool", bufs=1) as pool:
            for t in range(N_TILES):
                r0 = t * ROWS_PER_TILE
                xin = pool.tile([128, FLAT], F32, tag="xin", bufs=3)
                if t == 0:
                    # split the first load so tile-0 compute can start after
                    # the first 6 row-slots land instead of all 10
                    HALF0 = 6 * WP  # slots 0..5: inputs for the first 4 rows
                    nc.sync.dma_start(out=xin[:, 0:HALF0], in_=x_d[t][:, 0:HALF0])
                    nc.sync.dma_start(out=xin[:, HALF0:FLAT], in_=x_d[t][:, HALF0:FLAT])
                else:
                    nc.sync.dma_start(out=xin[:], in_=x_d[t])

                xf = xin[:]
                v0 = xf[:, 0:CLEN]
                v1 = xf[:, WP : WP + CLEN]
                v2 = xf[:, 2 * WP : 2 * WP + CLEN]

                P1 = pool.tile([128, CLEN], F32, tag="p1")
                P2 = pool.tile([128, CLEN], F32, tag="p2")
                S1 = pool.tile([128, CLEN], F32, tag="s1")
                S2 = pool.tile([128, CLEN], F32, tag="s2")
                S3 = pool.tile([128, CLEN], F32, tag="s3")
                T1 = pool.tile([128, CLEN], F32, tag="t1")
                O = pool.tile([128, CLEN], F32, tag="o", bufs=2)

                # column stage: per-position vertical min/med/max.
                # All xin readers stay on DVE so the input DMA's slot-reuse
                # wait collapses to one engine semaphore (DMA sync-wait
                # count is tightly limited in codegen).
                # For tile 0 the stage runs in two free-dim halves so the
                # first half starts as soon as the first input DMA lands.
                halves = ((0, 4 * WP), (4 * WP, CLEN)) if t == 0 else ((0, CLEN),)
                for lo, hi in halves:
                    h = slice(lo, hi)
                    vh0 = xf[:, lo:hi]
                    vh1 = xf[:, WP + lo : WP + hi]
                    vh2 = xf[:, 2 * WP + lo : 2 * WP + hi]
                    nc.vector.tensor_tensor(P1[:, h], vh0, vh1, MIN)  # pair min
                    nc.vector.tensor_tensor(P2[:, h], vh0, vh1, MAX)  # pair max
                    nc.vector.tensor_tensor(S1[:, h], P1[:, h], vh2, MIN)  # cmin
                    nc.vector.tensor_tensor(S2[:, h], P2[:, h], vh2, MAX)  # cmax
                    nc.vector.tensor_tensor(P2[:, h], P2[:, h], vh2, MIN)  # t5
                    nc.vector.tensor_tensor(S3[:, h], P1[:, h], P2[:, h], MAX)  # cmed

                c = slice(1, CLEN - 1)
                l = slice(0, CLEN - 2)
                r = slice(2, CLEN)

                # A = max3(cmin left, center, right)
                nc.vector.tensor_tensor(P1[:, c], S1[:, l], S1[:, r], MAX)
                nc.vector.tensor_tensor(P1[:, c], P1[:, c], S1[:, c], MAX)  # A
                # C = min3(cmax)
                nc.vector.tensor_tensor(T1[:, c], S2[:, l], S2[:, r], MIN)
                nc.vector.tensor_tensor(T1[:, c], T1[:, c], S2[:, c], MIN)  # C
                # B = med3(cmed l, c, r)
                nc.vector.tensor_tensor(S1[:, c], S3[:, l], S3[:, c], MIN)  # m1
                nc.vector.tensor_tensor(S2[:, c], S3[:, l], S3[:, c], MAX)  # m2
                nc.vector.tensor_tensor(S2[:, c], S2[:, c], S3[:, r], MIN)  # m3
                nc.vector.tensor_tensor(S1[:, c], S1[:, c], S2[:, c], MAX)  # B
                # out = med3(A=P1, B=S1, C=T1)
                nc.vector.tensor_tensor(S3[:, c], P1[:, c], S1[:, c], MIN)  # mn2
                nc.vector.tensor_tensor(P2[:, c], P1[:, c], S1[:, c], MAX)  # mx2
                nc.vector.tensor_tensor(P2[:, c], P2[:, c], T1[:, c], MIN)  # t3

                # median = max(mn2, t3); store rows y cols 1..512 of each
                # padded row. For the last tile, split the final op + store
                # so the first half of the output DMA overlaps the second
                # half of the compute.
                o3 = O[:].rearrange("p (s w) -> p s w", w=WP)
                dst = o_d[r0 : r0 + ROWS_PER_TILE].rearrange("(p s) w -> p s w", s=RPP)
                if t == N_TILES - 1:
                    mid = 4 * WP
                    nc.vector.tensor_tensor(
                        O[:, 1:mid], S3[:, 1:mid], P2[:, 1:mid], MAX
                    )
                    nc.sync.dma_start(out=dst[:, 0:4, :], in_=o3[:, 0:4, 1 : W + 1])
                    nc.vector.tensor_tensor(
                        O[:, mid : CLEN - 1], S3[:, mid : CLEN - 1],
                        P2[:, mid : CLEN - 1], MAX,
                    )
                    nc.sync.dma_start(out=dst[:, 4:RPP, :], in_=o3[:, 4:RPP, 1 : W + 1])
                else:
                    nc.vector.tensor_tensor(O[:, c], S3[:, c], P2[:, c], MAX)  # median
                    nc.sync.dma_start(out=dst, in_=o3[:, :, 1 : W + 1])
    return nc


def _get_nc():
    global _NC_CACHE
    if _NC_CACHE is None:
        nc = _build_bass()
        nc.compile()
        _NC_CACHE = nc
    return _NC_CACHE


def _stage_core(imgs):
    """imgs: [6, 512, 512] fp32 -> staged [3, 128, FLAT] with halo rows and
    reflect-padded rows/cols materialized."""
    xp = np.pad(imgs, ((0, 0), (1, 1), (1, 1)), mode="reflect")  # [6, 514, 514]
    # windows of 10 padded rows starting every 8 rows: [6, 64, 10, 514]
    win = np.lib.stride_tricks.sliding_window_view(xp, (NSLOT, WP), axis=(1, 2))
    blocks = win[:, ::RPP, 0]  # [6, 64, 10, 514]
    staged = blocks.reshape(N_TILES, 128, NSLOT, WP).reshape(N_TILES, 128, FLAT)
    return np.ascontiguousarray(staged)


def run(x, trace=False):
    """x: [16,3,512,512] fp32 -> (out [16,3,512,512] fp32, exec_time_ns|None)"""
    from concourse.bass_utils import run_bass_kernel_spmd

    x = np.ascontiguousarray(np.asarray(x, dtype=np.float32))
    B, C, H, Wd = x.shape
    imgs = x.reshape(8, 6, H, Wd)
    in_maps = [{"x": _stage_core(imgs[i])} for i in range(8)]
    nc = _get_nc()
    res = run_bass_kernel_spmd(nc, in_maps, list(range(8)), trace=trace)
    out = np.stack([res.results[i]["out"] for i in range(8)])
    return out.reshape(B, C, H, Wd), res.exec_time_ns


def kernel(x):
    out, _ = run(x, trace=False)
    return out



# revision 4
# speedup vs baseline: 2.3816x; 2.3816x over previous
"""MedianPool2d 3x3 stride-1 reflect-pad kernel for 8 TRN2 NeuronCores.

Input:  x [16, 3, 512, 512] fp32 (full). Output: same shape, lower median
of each 3x3 window after reflect pad. Computed in fp16 (tolerance 2e-2;
fp16 quantization contributes ~3e-4 norm-relative error).

Strategy:
 - Pure data parallel: 48 images (B*C) -> 6 images per core, no collectives.
 - fp16 + pair-interleaved layout: each tile holds TWO images with their
   columns interleaved (I[:, 2c] = A[:, c], I[:, 2c+1] = B[:, c]). A +-1
   column window shift is then a +-2 fp16 element offset = 4-byte aligned,
   so every tensor_tensor min/max qualifies for the DVE 2x_1P perf mode
   (16-bit dtype, step +-1, 4B-aligned). Vertical shifts are whole-slot
   offsets (1028 elems), also aligned.
 - Host staging: reflect pad to [514, 514], interleave pairs to [514, 1028],
   then per partition p slots = padded rows [4p, 4p+6) -> xin [128, 6168].
   All 9 window taps become free-dim offsets of one flat SBUF buffer.
 - Median-of-9 via med3(max3(col mins), med3(col meds), min3(col maxes)):
   18 full-tile min/max tensor_tensor ops.
 - Engine split: DVE computes flat columns [0, DS), GpSimd (Pool) columns
   [DS, 4112) with a private 2-element-halo stat region - fully independent
   chains, no cross-engine semaphores except the DMAs.
 - Output stays interleaved fp16 in DRAM; host de-interleaves + upcasts.
"""

import sys

for _p in ("/opt/trn_rl_repo", "/root/.axon_site/_ro/trn_rl_repo"):
    if _p not in sys.path:
        sys.path.append(_p)

import numpy as np

import concourse.bass as bass
import concourse.bacc as bacc
import concourse.mybir as mybir
from concourse.tile import TileContext

F16 = mybir.dt.float16
MIN = mybir.AluOpType.min
MAX = mybir.AluOpType.max

W = 512
WP2 = 1028           # interleaved padded pair-row width (2 * 514)
RPP = 4              # pair-rows per partition
NSLOT = RPP + 2      # + top/bottom halo rows
FLAT2 = NSLOT * WP2  # 6168 fp16 per partition staged input
CLEN2 = RPP * WP2    # 4112 flat stat/output length
N_TILES = 3          # one image-pair per tile
USE_POOL = False     # GpSimd offload (walrus rejects TensorTensor on Pool)
DS = 3370            # DVE handles flat [0, DS), Pool [DS, CLEN2). Even!


def _emit_chain(nc, eng, pool, xin, tag, base, L, out_lo, out_hi, o_tile, t,
                col_splits=None):
    """Emit the 18-op median chain on `eng` for global flat range:
    col stats over [base, base+L), row outputs over [out_lo, out_hi).
    Stats buffers are private (tagged); indices into them are global-base.
    o_tile: output tile buffer (length L, local coords = global - base).
    """
    P1 = pool.tile([128, L], F16, tag=f"{tag}p1")
    P2 = pool.tile([128, L], F16, tag=f"{tag}p2")
    S1 = pool.tile([128, L], F16, tag=f"{tag}s1")
    S2 = pool.tile([128, L], F16, tag=f"{tag}s2")
    S3 = pool.tile([128, L], F16, tag=f"{tag}s3")
    T1 = pool.tile([128, L], F16, tag=f"{tag}t1")

    # column stage: vertical min/med/max per flat position
    splits = col_splits if col_splits else (L,)
    lo = 0
    for hi in splits:
        h = slice(lo, hi)
        v0 = xin[:, base + lo : base + hi]
        v1 = xin[:, WP2 + base + lo : WP2 + base + hi]
        v2 = xin[:, 2 * WP2 + base + lo : 2 * WP2 + base + hi]
        eng.tensor_tensor(P1[:, h], v0, v1, MIN)
        eng.tensor_tensor(P2[:, h], v0, v1, MAX)
        eng.tensor_tensor(S1[:, h], P1[:, h], v2, MIN)  # cmin
        eng.tensor_tensor(S2[:, h], P2[:, h], v2, MAX)  # cmax
        eng.tensor_tensor(P2[:, h], P2[:, h], v2, MIN)
        eng.tensor_tensor(S3[:, h], P1[:, h], P2[:, h], MAX)  # cmed
        lo = hi

    # row stage over local out range
    a, b = out_lo - base, out_hi - base
    c = slice(a, b)
    l = slice(a - 2, b - 2)
    r = slice(a + 2, b + 2)
    eng.tensor_tensor(P1[:, c], S1[:, l], S1[:, r], MAX)
    eng.tensor_tensor(P1[:, c], P1[:, c], S1[:, c], MAX)   # A = max3(cmin)
    eng.tensor_tensor(T1[:, c], S2[:, l], S2[:, r], MIN)
    eng.tensor_tensor(T1[:, c], T1[:, c], S2[:, c], MIN)   # C = min3(cmax)
    eng.tensor_tensor(S1[:, c], S3[:, l], S3[:, c], MIN)
    eng.tensor_tensor(S2[:, c], S3[:, l], S3[:, c], MAX)
    eng.tensor_tensor(S2[:, c], S2[:, c], S3[:, r], MIN)
    eng.tensor_tensor(S1[:, c], S1[:, c], S2[:, c], MAX)   # B = med3(cmed)
    eng.tensor_tensor(S3[:, c], P1[:, c], S1[:, c], MIN)
    eng.tensor_tensor(P2[:, c], P1[:, c], S1[:, c], MAX)
    eng.tensor_tensor(P2[:, c], P2[:, c], T1[:, c], MIN)
    eng.tensor_tensor(o_tile[:, c], S3[:, c], P2[:, c], MAX)  # median


def _build_bass(loop_k=1):
    nc = bacc.Bacc("TRN2", target_bir_lowering=False)
    x_d = nc.declare_dram_parameter("x", [N_TILES, 128, FLAT2], F16, isOutput=False)
    o_d = nc.declare_dram_parameter("out", [N_TILES, 128, CLEN2], F16, isOutput=True)

    PB = DS - 2          # pool stats base
    PL = CLEN2 - PB      # pool stats length (incl. 2-halo)

    import contextlib
    with TileContext(nc) as tc:
        loop_cm = tc.For_i(0, loop_k, 1) if loop_k > 1 else contextlib.nullcontext()
        with loop_cm, tc.tile_pool(name="pool", bufs=1) as pool:
            for t in range(N_TILES):
                xin = pool.tile([128, FLAT2], F16, tag="xin", bufs=3)
                if t == 0:
                    # split first load so compute starts after slots 0-3 land
                    HALF0 = 4 * WP2
                    nc.sync.dma_start(out=xin[:, 0:HALF0], in_=x_d[t][:, 0:HALF0])
                    nc.sync.dma_start(out=xin[:, HALF0:FLAT2], in_=x_d[t][:, HALF0:FLAT2])
                    dve_splits = (2 * WP2, DS + 2)
                else:
                    nc.sync.dma_start(out=xin[:], in_=x_d[t])
                    dve_splits = None

                if USE_POOL:
                    O = pool.tile([128, DS], F16, tag="odve", bufs=2)
                    OP = pool.tile([128, PL], F16, tag="opool", bufs=2)

                    # DVE: global [0, DS+2) stats, outputs [2, DS)
                    _emit_chain(nc, nc.vector, pool, xin, "d", 0, DS + 2, 2, DS,
                                O, t, col_splits=dve_splits)
                    nc.sync.dma_start(out=o_d[t][:, 2:DS], in_=O[:, 2:DS])

                    # Pool: global [DS-2, CLEN2) stats, outputs [DS, CLEN2-2)
                    _emit_chain(nc, nc.gpsimd, pool, xin, "q", PB, PL,
                                DS, CLEN2 - 2, OP, t)
                    nc.sync.dma_start(out=o_d[t][:, DS : CLEN2 - 2],
                                      in_=OP[:, 2 : PL - 2])
                else:
                    O = pool.tile([128, CLEN2], F16, tag="odve", bufs=2)
                    if dve_splits is not None:
                        dve_splits = (2 * WP2, CLEN2)
                    _emit_chain(nc, nc.vector, pool, xin, "d", 0, CLEN2,
                                2, CLEN2 - 2, O, t, col_splits=dve_splits)
                    nc.sync.dma_start(out=o_d[t][:, 2 : CLEN2 - 2],
                                      in_=O[:, 2 : CLEN2 - 2])
    return nc


_NC_CACHE = None


def _get_nc():
    global _NC_CACHE
    if _NC_CACHE is None:
        nc = _build_bass()
        nc.compile()
        _NC_CACHE = nc
    return _NC_CACHE


def _stage_core(imgs):
    """imgs: [6, 512, 512] float -> staged [3, 128, FLAT2] fp16: pairs of
    images reflect-padded, column-interleaved, 6-row sliding slots."""
    imgs = np.asarray(imgs, dtype=np.float16)
    xp = np.pad(imgs, ((0, 0), (1, 1), (1, 1)), mode="reflect")  # [6, 514, 514]
    inter = np.empty((N_TILES, 514, WP2), dtype=np.float16)
    inter[:, :, 0::2] = xp[0::2]
    inter[:, :, 1::2] = xp[1::2]
    # partition p of tile t: rows [4p, 4p+6)
    idx = np.arange(128)[:, None] * RPP + np.arange(NSLOT)[None, :]  # [128, 6]
    blocks = inter[:, idx, :]  # [3, 128, 6, 1028]
    return np.ascontiguousarray(blocks.reshape(N_TILES, 128, FLAT2))


def _unstage_core(out_d):
    """out_d: [3, 128, CLEN2] fp16 -> [6, 512, 512] fp32."""
    o = out_d.reshape(N_TILES, 128, RPP, WP2)[:, :, :, 2 : 2 + 2 * W]
    o = o.reshape(N_TILES, 512, 2 * W)
    res = np.empty((6, 512, 512), dtype=np.float32)
    res[0::2] = o[:, :, 0::2].astype(np.float32)
    res[1::2] = o[:, :, 1::2].astype(np.float32)
    return res


def run(x, trace=False):
    """x: [16,3,512,512] fp32 -> (out [16,3,512,512] fp32, exec_time_ns|None)"""
    from concourse.bass_utils import run_bass_kernel_spmd

    x = np.ascontiguousarray(np.asarray(x, dtype=np.float32))
    B, C, H, Wd = x.shape
    imgs = x.reshape(8, 6, H, Wd)
    in_maps = [{"x": _stage_core(imgs[i])} for i in range(8)]
    nc = _get_nc()
    res = run_bass_kernel_spmd(nc, in_maps, list(range(8)), trace=trace)
    out = np.stack([_unstage_core(res.results[i]["out"]) for i in range(8)])
    return out.reshape(B, C, H, Wd), res.exec_time_ns


def kernel(x):
    out, _ = run(x, trace=False)
    return out
